# revision 1
# baseline (speedup 1.0000x reference)
"""RBF/KNN interpolation kernel for Trainium2 (8 NeuronCores, data parallel).

Computes, per batch b:
    v        = input_data[b, -1, :, 0]                      (N_in,)
    w[o, i]  = exp(-||tc[o] - ic[i]||^2 / (2 * 0.1^2))      (N_out, N_in)
    interp   = (w @ v) / (w.sum(-1) + 1e-8)                 (N_out,)
    out[b]   = broadcast(interp) -> (n_samples, N_out, 4)

Sharding: batch B=8 across 8 cores (one batch per core). The weight matrix
is built on-chip tile by tile (never materialized in HBM):
  - logits psum[i, o] via a K=8 fp16 matmul. fp32 coords are split into
    fp16 (hi, lo) pairs so the single-pass fp16 PE path keeps ~1e-4
    precision on the exponent (fp32 matmul runs 2 passes at half rate):
      cross = xh*txh + xh*txl + xl*txh + (same for y) + 1*t2h + 1*t2l
    where t2h + t2l ~= -0.5 * |tc|^2.
  - w = Exp(100 * logits + bias[i]) on the scalar engine, written as fp16;
    bias = -50*|ic|^2 + 10*ln(2) (the 2^10 factor keeps small weights out
    of the fp16 denormal range; it cancels in num/den).
  - [num; ...; den] += [v, 0 x31, 1].T @ w  (fp16 matmul, fp32 psum accum;
    den lands on psum partition 32 - compute-engine PSUM APs need 32-aligned
    starts).
  - interp = num / (den + 1024e-8), computed in a [128, L] layout, then
    broadcast x4 (vector copies) and x n_samples (DMA) to the output.
"""

from contextlib import ExitStack
from functools import lru_cache

import numpy as np

import concourse.bass as bass
import concourse.bacc as bacc
import concourse.tile as tile
from concourse import mybir
from concourse.bass_utils import run_bass_kernel_spmd

F32 = mybir.dt.float32
F16 = mybir.dt.float16
AF = mybir.ActivationFunctionType
ALU = mybir.AluOpType

# Problem sizes (hardcoded per spec)
B = 8
T_IN = 4
N_IN = 4096
V_IN = 3
N_OUT = 8192
S = 10
T_OUT = 4
GAMMA = 50.0  # 1 / (2 * LENGTH_SCALE^2), LENGTH_SCALE = 0.1
EPS = 1e-8
WSCALE_LOG = 6.93147180559945  # ln(2^10)
WSCALE = 1024.0


def build_kernel(tc_ctx, dat, ic_h, tc_h, out_h, n_in, n_out, s, F=1024):
    tcx = tc_ctx
    nc = tcx.nc
    IC = n_in // 128   # i-chunks
    OC = n_out // F    # o-chunks
    NSUB = F // 512
    L = n_out // 128   # per-partition interp count in output layout
    CT = n_out // 128  # nat-layout columns (target side)

    with ExitStack() as ctx:
        const_pool = ctx.enter_context(tcx.tile_pool(name="const", bufs=1))

        # ---- persistent tiles ----
        # K is zero-padded 8 -> 128: a full-array matmul costs the same cycles
        # (stream rate is per column) but keeps the PE HAM activity monitor
        # seeing a busy array, so the clock un-throttles to 2.4 GHz.
        tc_aug = const_pool.tile([128, n_out], F16)  # rows t2h t2l txh txl txh tyh tyl tyh, rest 0
        ic_aug = const_pool.tile([128, n_in], F16)   # rows 1   1   xh  xh  xl  yh  yh  yl, rest 0
        bias_nat = const_pool.tile([128, IC], F32)
        vo_nat = const_pool.tile([128, 128 * IC], F16)  # [v, 0..., 1@32, 0...] per chunk
        nd_rows = const_pool.tile([33, n_out], F32)  # row 0 = num, row 32 = den
        ident = const_pool.tile([128, 128], F16)

        # ---- head: inputs, identity, splits (all in 128-partition nat layout) ----
        head = ctx.enter_context(tcx.tile_pool(name="head", bufs=1))
        tcx_nat = head.tile([128, CT], F32)
        tcy_nat = head.tile([128, CT], F32)
        icx_nat = head.tile([128, IC], F32)
        icy_nat = head.tile([128, IC], F32)
        v_nat = head.tile([128, IC], F32)

        # PE clock warm-up: the HAM monitor un-throttles (1.2 -> 2.4 GHz)
        # only after ~3.4us of sustained full-array work and re-throttles
        # after ~3.4us idle. Fill the PE's head idle time with dummy
        # full-array matmuls whose source is ready immediately, sized to end
        # roughly when the transpose inputs become ready.
        warm_cm = tcx.tile_pool(name="warm_ps", bufs=2, space="PSUM")
        warm_ps = warm_cm.__enter__()
        wsrc = head.tile([128, 512], F16)
        nc.gpsimd.memset(wsrc[:, :].bitcast(mybir.dt.uint32), 0)
        for _ in range(55):
            wps = warm_ps.tile([128, 512], F32, tag="warm")
            nc.tensor.matmul(wps[:, :], wsrc[:, 0:128], wsrc[:, :],
                             start=True, stop=True)

        # zero the padded-K operands first (they gate the row DMAs);
        # bitcast fp16 pairs to uint32 to halve the element count
        tc_aug_u = tc_aug[:, :].bitcast(mybir.dt.uint32)
        ic_aug_u = ic_aug[:, :].bitcast(mybir.dt.uint32)
        nc.vector.memset(tc_aug_u[:, :tc_aug_u.shape[1] // 2], 0)
        nc.gpsimd.memset(tc_aug_u[:, tc_aug_u.shape[1] // 2:], 0)
        nc.vector.memset(ic_aug_u, 0)

        # coordinate loads, one contiguous nat tile per component
        tc_r = tc_h[:].rearrange("(c p) d -> p c d", p=128)
        ic_r = ic_h[:].rearrange("(c p) d -> p c d", p=128)
        h = CT // 2
        nc.sync.dma_start(out=tcx_nat[:, 0:h].rearrange("p (c o) -> p c o", o=1),
                          in_=tc_r[:, 0:h, 0:1])
        nc.gpsimd.dma_start(out=tcx_nat[:, h:].rearrange("p (c o) -> p c o", o=1),
                            in_=tc_r[:, h:, 0:1])
        nc.scalar.dma_start(out=tcy_nat[:, 0:h].rearrange("p (c o) -> p c o", o=1),
                            in_=tc_r[:, 0:h, 1:2])
        nc.sync.dma_start(out=tcy_nat[:, h:].rearrange("p (c o) -> p c o", o=1),
                          in_=tc_r[:, h:, 1:2])
        nc.gpsimd.dma_start(out=icx_nat.rearrange("p (c o) -> p c o", o=1),
                            in_=ic_r[:, :, 0:1])
        nc.scalar.dma_start(out=icy_nat.rearrange("p (c o) -> p c o", o=1),
                            in_=ic_r[:, :, 1:2])
        nc.sync.dma_start(
            out=v_nat[:, :],
            in_=dat[:][T_IN - 1, :, 0].rearrange("(c p) -> p c", p=128),
        )

        # identity for PE transposes: ident[p, f] = (p == f)
        jj = head.tile([128, 128], F32)
        kk = head.tile([128, 1], F32)
        nc.gpsimd.iota(jj[:, :], [[1, 128]], base=0, channel_multiplier=0,
                       allow_small_or_imprecise_dtypes=True)
        nc.gpsimd.iota(kk[:, :], [[0, 1]], base=0, channel_multiplier=1,
                       allow_small_or_imprecise_dtypes=True)
        nc.gpsimd.tensor_scalar(ident[:, :], jj[:, :], kk[:, 0:1], None,
                                op0=ALU.is_equal)

        # --- target-side nat computes (chunk-major: x[p, c] = f(tc[c*128+p])) ---
        sqx_t = head.tile([128, CT], F32)
        t2s = head.tile([128, CT], F32)
        nc.gpsimd.tensor_mul(sqx_t[:, :], tcx_nat[:, :], tcx_nat[:, :])
        nc.vector.scalar_tensor_tensor(t2s[:, :], tcy_nat[:, :], 1.0,
                                       tcy_nat[:, :], op0=ALU.bypass,
                                       op1=ALU.mult)
        nc.vector.tensor_add(t2s[:, :], t2s[:, :], sqx_t[:, :])

        t2h_nat = head.tile([128, CT], F16)
        t2l_nat = head.tile([128, CT], F16)
        nc.vector.tensor_scalar_mul(t2h_nat[:, :], t2s[:, :], -0.5)
        nc.vector.scalar_tensor_tensor(t2l_nat[:, :], t2s[:, :], -0.5,
                                       t2h_nat[:, :], op0=ALU.mult,
                                       op1=ALU.subtract)
        txh_nat = head.tile([128, CT], F16)
        txl_nat = head.tile([128, CT], F16)
        tyh_nat = head.tile([128, CT], F16)
        tyl_nat = head.tile([128, CT], F16)
        nc.gpsimd.tensor_copy(txh_nat[:, :], tcx_nat[:, :])
        nc.vector.tensor_sub(txl_nat[:, :], tcx_nat[:, :], txh_nat[:, :])
        nc.gpsimd.tensor_copy(tyh_nat[:, :], tcy_nat[:, :])
        nc.vector.tensor_sub(tyl_nat[:, :], tcy_nat[:, :], tyh_nat[:, :])

        # --- input-side nat computes ---
        sqx_i = head.tile([128, IC], F32)
        i2s = head.tile([128, IC], F32)
        nc.gpsimd.tensor_mul(sqx_i[:, :], icx_nat[:, :], icx_nat[:, :])
        nc.vector.scalar_tensor_tensor(i2s[:, :], icy_nat[:, :], 1.0,
                                       icy_nat[:, :], op0=ALU.bypass,
                                       op1=ALU.mult)
        nc.vector.tensor_add(i2s[:, :], i2s[:, :], sqx_i[:, :])
        # bias = -50 * i2 + ln(2^10)
        nc.vector.tensor_scalar(bias_nat[:, :], i2s[:, :], -GAMMA, WSCALE_LOG,
                                op0=ALU.mult, op1=ALU.add)

        xh_nat = head.tile([128, IC], F16)
        xl_nat = head.tile([128, IC], F16)
        yh_nat = head.tile([128, IC], F16)
        yl_nat = head.tile([128, IC], F16)
        nc.gpsimd.tensor_copy(xh_nat[:, :], icx_nat[:, :])
        nc.vector.tensor_sub(xl_nat[:, :], icx_nat[:, :], xh_nat[:, :])
        nc.gpsimd.tensor_copy(yh_nat[:, :], icy_nat[:, :])
        nc.vector.tensor_sub(yl_nat[:, :], icy_nat[:, :], yh_nat[:, :])

        # vo_nat: col 128c = v (fp16), col 128c+32 = 1.0, rest 0 (M padded to 128)
        nc.gpsimd.memset(vo_nat[:, :].bitcast(mybir.dt.uint32), 0)
        vo3 = vo_nat.rearrange("p (c w) -> p c w", w=128)
        nc.vector.tensor_copy(vo3[:, :, 0], v_nat[:, :])
        nc.vector.memset(vo3[:, :, 32], 1.0)

        # --- nat -> row layout via PE transpose + copy + DMA ---
        with tcx.tile_pool(name="tps", bufs=2, space="PSUM") as tp_pool, \
             tcx.tile_pool(name="tsb", bufs=2) as tsb_pool:

            tp_count = [0]

            def to_rows(nat, ncols, aug, rows):
                ps = tp_pool.tile([128, 128], F16, tag="ps")
                sb = tsb_pool.tile([128, 128], F16, tag="sb")
                nc.tensor.transpose(ps[:ncols, :], nat[:, :], ident[:, :])
                if tp_count[0] % 2 == 0:
                    nc.vector.tensor_copy(sb[:ncols, :], ps[:ncols, :])
                else:
                    nc.scalar.copy(sb[:ncols, :], ps[:ncols, :])
                tp_count[0] += 1
                for r in rows:
                    nc.sync.dma_start(
                        out=aug[r:r + 1, :].rearrange("r (c p) -> r c p", p=128),
                        in_=sb[:ncols, :],
                    )

            to_rows(t2h_nat, CT, tc_aug, [0])
            to_rows(t2l_nat, CT, tc_aug, [1])
            to_rows(txh_nat, CT, tc_aug, [2, 4])
            to_rows(txl_nat, CT, tc_aug, [3])
            to_rows(tyh_nat, CT, tc_aug, [5, 7])
            to_rows(tyl_nat, CT, tc_aug, [6])

            # rows 0,1 = 1.0: 0x3C00 fp16 pairs as uint32
            nc.vector.memset(ic_aug[0:2, :].bitcast(mybir.dt.uint32), 0x3C003C00)
            to_rows(xh_nat, IC, ic_aug, [2, 3])
            to_rows(xl_nat, IC, ic_aug, [4])
            to_rows(yh_nat, IC, ic_aug, [5, 6])
            to_rows(yl_nat, IC, ic_aug, [7])
            # bridge the transposes -> first-matmul window so HAM stays warm
            for _ in range(20):
                wps = warm_ps.tile([128, 512], F32, tag="warm")
                nc.tensor.matmul(wps[:, :], wsrc[:, 0:128], wsrc[:, :],
                                 start=True, stop=True)

        warm_cm.__exit__(None, None, None)

        # ---- main loop ----
        # F=1024 is the psum sweet spot on trn2: pl [128,1024] f32 = 2 banks
        # x2 bufs + nd [128,1024] x2 bufs = 8 banks total. Every 2 o-chunks we
        # finalize a 32-partition output group (divide + x4 + x10 broadcast)
        # so the tail overlaps the main loop.
        PG = 2 * F // L  # output partitions finalized per 2 o-chunks
        with (
            tcx.tile_pool(name="psum_l", bufs=2, space="PSUM") as pl_pool,
            tcx.tile_pool(name="psum_nd", bufs=2, space="PSUM") as nd_pool,
            tcx.tile_pool(name="w", bufs=3) as w_pool,
            tcx.tile_pool(name="grp", bufs=2) as grp_pool,
        ):
            for oc in range(OC):
                nd = nd_pool.tile([128, F], F32)
                for icc in range(IC):
                    pl = pl_pool.tile([128, F], F32)
                    lhsT1 = ic_aug[:, icc * 128:(icc + 1) * 128]
                    for sub in range(NSUB):
                        nc.tensor.matmul(
                            pl[:, sub * 512:(sub + 1) * 512],
                            lhsT1,
                            tc_aug[:, oc * F + sub * 512: oc * F + (sub + 1) * 512],
                            start=True,
                            stop=True,
                        )
                    w = w_pool.tile([128, F], F16)
                    nc.scalar.activation(
                        w[:, :],
                        pl[:, :],
                        AF.Exp,
                        bias=bias_nat[:, icc:icc + 1],
                        scale=2.0 * GAMMA,
                    )
                    for sub in range(NSUB):
                        nc.tensor.matmul(
                            nd[:, sub * 512:(sub + 1) * 512],
                            vo_nat[:, 128 * icc:128 * icc + 128],
                            w[:, sub * 512:(sub + 1) * 512],
                            start=(icc == 0),
                            stop=(icc == IC - 1),
                        )
                nc.vector.tensor_copy(nd_rows[0:33, oc * F:(oc + 1) * F],
                                      nd[0:33, :])
                # finalize this o-chunk's output partitions right away so
                # only the last chunk's finalize is exposed past the loop
                PG2 = F // L
                c0, c1 = oc * F, (oc + 1) * F
                gnum = grp_pool.tile([PG2, L], F32, tag="gnum")
                gden = grp_pool.tile([PG2, L], F32, tag="gden")
                grep = grp_pool.tile([PG2, 4 * L], F32, tag="grep")
                nc.sync.dma_start(
                    out=gnum[:, :],
                    in_=nd_rows[0:1, c0:c1].rearrange("r (p k) -> r p k", k=L),
                )
                nc.gpsimd.dma_start(
                    out=gden[:, :],
                    in_=nd_rows[32:33, c0:c1].rearrange("r (p k) -> r p k", k=L),
                )
                nc.vector.tensor_scalar_add(gden[:, :], gden[:, :], EPS * WSCALE)
                nc.vector.reciprocal(gden[:, :], gden[:, :])
                nc.vector.tensor_mul(gnum[:, :], gnum[:, :], gden[:, :])
                grep3 = grep.rearrange("p (k t) -> p k t", t=4)
                for t in range(4):
                    nc.vector.tensor_copy(grep3[:, :, t], gnum[:, :])
                engs = [nc.sync, nc.gpsimd]
                for si in range(s):
                    engs[si % len(engs)].dma_start(
                        out=out_h[:][si].rearrange("o t -> (o t)").rearrange(
                            "(p j) -> p j", p=n_out * 4 // (4 * L))[
                                oc * PG2:(oc + 1) * PG2, :],
                        in_=grep[:, :],
                    )


@lru_cache(maxsize=2)
def build_nc(n_in=N_IN, n_out=N_OUT, s=S, F=1024):
    nc = bacc.Bacc("TRN2", target_bir_lowering=False, debug=False)
    dat = nc.dram_tensor("dat", [T_IN, n_in, V_IN], F32, kind="ExternalInput")
    ic_h = nc.dram_tensor("ic", [n_in, 2], F32, kind="ExternalInput")
    tc_h = nc.dram_tensor("tc", [n_out, 2], F32, kind="ExternalInput")
    out_h = nc.dram_tensor("out", [s, n_out, T_OUT], F32, kind="ExternalOutput")
    with tile.TileContext(nc) as tcx:
        build_kernel(tcx, dat, ic_h, tc_h, out_h, n_in, n_out, s, F=F)
    nc.compile()
    return nc


def _run(input_data, input_coords, target_coords, n_samples, trace=False):
    n_samples = int(n_samples)
    assert n_samples == S, f"kernel compiled for n_samples={S}, got {n_samples}"
    assert input_data.shape == (B, T_IN, N_IN, V_IN)
    nc = build_nc()
    in_maps = [
        {
            "dat": np.ascontiguousarray(input_data[b], dtype=np.float32),
            "ic": np.ascontiguousarray(input_coords[b], dtype=np.float32),
            "tc": np.ascontiguousarray(target_coords[b], dtype=np.float32),
        }
        for b in range(B)
    ]
    res = run_bass_kernel_spmd(nc, in_maps, list(range(B)), trace=trace)
    out = np.stack([res.results[b]["out"] for b in range(B)], axis=0)
    return out, res


def kernel(input_data, input_coords, target_coords, n_samples):
    out, _ = _run(
        np.asarray(input_data),
        np.asarray(input_coords),
        np.asarray(target_coords),
        n_samples,
    )
    return out



# revision 7
# speedup vs baseline: 1.1569x; 1.1569x over previous
"""RBF/KNN interpolation kernel for Trainium2 (8 NeuronCores, data parallel).

Algorithmic core: the Gaussian RBF kernel is separable and effectively
low-rank on [0,1]^2.  With sigma = 0.1,

    exp(-(a-b)^2 / (2 s^2)) = sum_m  a_m cos(pi m (a-b)),
    a_m = s sqrt(2 pi) exp(-(pi m s)^2 / 2)   (a_0 halved),

truncated at m <= 12 (error ~2e-5).  Expanding cos(pi m (a-b)) into
cos/sin products gives a rank-25-per-dimension feature map phi, and the
2-D kernel w[o,i] = kx * ky becomes a bilinear form:

    interp(o) = [phix(t_o)^T M1 phiy(t_o)] / [phix(t_o)^T M0 phiy(t_o)],
    M1 = sum_i v_i phix(x_i) phiy(x_i)^T,   M0 = same with v=1.

So instead of an 8192x4096 dense weight pass (33.5M exps), each core does
a few small matmuls over 64 feature slots:

  1. args u = (m/2) t + phase via a K=8 PE outer product (fp16 hi/lo
     coord split; freqs m/2 are exactly fp16-representable),
  2. range-reduce u to [-0.5, 0.5) turns with the fp32 magic-number
     round trick (HW Sin is only accurate on [-pi, pi]),
  3. one Sin activation -> 64 feature rows per 512-target tile,
  4. G = lhsG^T ft  (lhsG folds a_n and M1/M0), T = ft .* G,
     num/den = redT^T T (redT folds a_m) -- all PE matmuls,
  5. divide in a [16, 64] nat layout, broadcast x4 and x n_samples
     to the output (same tail as the dense version).

Slot layout (64): 0:13 x-cos m=0..12, 16:29 y-cos, 32:44 x-sin m=1..12,
48:60 y-sin, rest zero.  Coefficients enter only through the small
matmul operands (lhsG per-n, redT per-m), never the big feature tiles.
"""

from contextlib import ExitStack
from functools import lru_cache

import numpy as np

import concourse.bass as bass
import concourse.bacc as bacc
import concourse.tile as tile
from concourse import mybir
from concourse.bass_utils import run_bass_kernel_spmd

F32 = mybir.dt.float32
F16 = mybir.dt.float16
AF = mybir.ActivationFunctionType
ALU = mybir.AluOpType

B = 8
T_IN = 4
N_IN = 4096
V_IN = 3
N_OUT = 8192
S = 10
T_OUT = 4
SIG = 0.1
EPS = 1e-8
MM = 12            # max cosine harmonic
MAGIC = 12582912.0  # 1.5 * 2^23: x + MAGIC - MAGIC == round(x) for |x| < 2^22
TWO_PI = 2.0 * np.pi

XCOS = list(range(0, 13))
YCOS = list(range(16, 29))
XSIN = list(range(32, 44))
YSIN = list(range(48, 60))


def _consts():
    def am(m):
        v = SIG * np.sqrt(2 * np.pi) * np.exp(-((np.pi * m * SIG) ** 2) / 2)
        return v / 2 if m == 0 else v

    cmat = np.zeros((8, 128), np.float16)
    for i, m in enumerate(range(0, MM + 1)):
        cmat[0, XCOS[i]] = m / 2.0
        cmat[2, XCOS[i]] = m / 2.0
        cmat[4, XCOS[i]] = 0.25
        cmat[1, YCOS[i]] = m / 2.0
        cmat[3, YCOS[i]] = m / 2.0
        cmat[4, YCOS[i]] = 0.25
    for i, m in enumerate(range(1, MM + 1)):
        cmat[0, XSIN[i]] = m / 2.0
        cmat[2, XSIN[i]] = m / 2.0
        cmat[1, YSIN[i]] = m / 2.0
        cmat[3, YSIN[i]] = m / 2.0

    agy = np.zeros((64, 1), np.float32)
    for i, m in enumerate(range(0, MM + 1)):
        agy[YCOS[i], 0] = am(m)
    for i, m in enumerate(range(1, MM + 1)):
        agy[YSIN[i], 0] = am(m)

    redt = np.zeros((128, 128), np.float16)
    for i, m in enumerate(range(0, MM + 1)):
        redt[XCOS[i], 0] = am(m)
        redt[64 + XCOS[i], 32] = am(m)
    for i, m in enumerate(range(1, MM + 1)):
        redt[XSIN[i], 0] = am(m)
        redt[64 + XSIN[i], 32] = am(m)

    ident = np.eye(128, dtype=np.float16)
    return cmat, agy, redt, ident


def build_kernel(tcx, dat, ic_h, tc_h, cm_h, ag_h, rt_h, id_h, on_h, mg_h, out_h):
    nc = tcx.nc
    IC = N_IN // 128
    F = 512
    NT = N_OUT // F        # main-loop column tiles
    NTI = N_IN // F        # input-side column tiles
    L = N_OUT // 128       # targets per output partition
    PG2 = 2 * F // L       # output partitions finalized per 2 tiles

    with ExitStack() as ctx:
        const_pool = ctx.enter_context(tcx.tile_pool(name="const", bufs=1))
        cmat = const_pool.tile([8, 128], F16)
        agy = const_pool.tile([64, 1], F32)
        mgc = const_pool.tile([64, 1], F32)
        redt = const_pool.tile([128, 128], F16)
        ident = const_pool.tile([128, 128], F16)
        trows = const_pool.tile([8, N_OUT], F16)
        irows = const_pool.tile([8, N_IN], F16)
        v_nat = const_pool.tile([128, IC], F32)
        fin_rows = const_pool.tile([64, N_IN], F16)
        fin_nat = const_pool.tile([128, IC * 128], F16)
        msb = const_pool.tile([128, 64], F16)
        lhsg = const_pool.tile([128, 128], F16)
        nd_rows = const_pool.tile([33, N_OUT], F32)

        psa_cm = tcx.tile_pool(name="psa", bufs=2, space="PSUM")
        psa_pool = psa_cm.__enter__()
        psg_cm = tcx.tile_pool(name="psg", bufs=2, space="PSUM")
        psg_pool = psg_cm.__enter__()
        psm_cm = tcx.tile_pool(name="psm", bufs=1, space="PSUM")
        psm_pool = psm_cm.__enter__()

        # PE clock warm-up (HAM un-throttles only under sustained
        # full-array work); fills the head DMA window.
        wsrc = const_pool.tile([128, 512], F16)
        nc.gpsimd.memset(wsrc[:, :].bitcast(mybir.dt.uint32), 0)
        for _ in range(30):
            wps = psg_pool.tile([128, F], F32, tag="psg")
            nc.tensor.matmul(wps[:, :], wsrc[:, 0:128], wsrc[:, :],
                             start=True, stop=True)

        stage_cm = tcx.tile_pool(name="stage", bufs=1)
        stage = stage_cm.__enter__()
        txy = stage.tile([2, N_OUT], F32)
        ixy = stage.tile([2, N_IN], F32)
        tlo = stage.tile([2, N_OUT], F16)
        ilo = stage.tile([2, N_IN], F16)

        # ---- head DMAs ----
        nc.sync.dma_start(out=cmat[:, :], in_=cm_h[:])
        nc.sync.dma_start(out=agy[:, :], in_=ag_h[:])
        nc.sync.dma_start(out=mgc[:, :], in_=mg_h[:])
        nc.gpsimd.dma_start(out=redt[:, :], in_=rt_h[:])
        nc.gpsimd.dma_start(out=ident[:, :], in_=id_h[:])
        nc.scalar.dma_start(out=ixy[0:1, :].rearrange("r (c o) -> r c o", o=1),
                            in_=ic_h[:][:, 0:1])
        nc.scalar.dma_start(out=ixy[1:2, :].rearrange("r (c o) -> r c o", o=1),
                            in_=ic_h[:][:, 1:2])
        nc.scalar.dma_start(
            out=v_nat[:, :],
            in_=dat[:][T_IN - 1, :, 0].rearrange("(c p) -> p c", p=128),
        )
        h = N_OUT // 2
        nc.sync.dma_start(out=txy[0:1, 0:h].rearrange("r (c o) -> r c o", o=1),
                          in_=tc_h[:][0:h, 0:1])
        nc.sync.dma_start(out=txy[0:1, h:].rearrange("r (c o) -> r c o", o=1),
                          in_=tc_h[:][h:, 0:1])
        nc.gpsimd.dma_start(out=txy[1:2, 0:h].rearrange("r (c o) -> r c o", o=1),
                            in_=tc_h[:][0:h, 1:2])
        nc.gpsimd.dma_start(out=txy[1:2, h:].rearrange("r (c o) -> r c o", o=1),
                            in_=tc_h[:][h:, 1:2])

        # ---- hi/lo fp16 coordinate splits ----
        # compute engines need 32-aligned partition starts: hi rows write
        # at offset 0 directly; lo rows go via staging tile + DMA
        nc.vector.memset(trows[:, :].bitcast(mybir.dt.uint32), 0)
        nc.gpsimd.memset(irows[:, :].bitcast(mybir.dt.uint32), 0)
        nc.vector.tensor_copy(irows[0:2, :], ixy[:, :])
        nc.vector.tensor_sub(ilo[:, :], ixy[:, :], irows[0:2, :])
        nc.gpsimd.tensor_copy(trows[0:2, :], txy[:, :])
        nc.vector.tensor_sub(tlo[:, :], txy[:, :], trows[0:2, :])
        nc.vector.memset(lhsg[64:128, :].bitcast(mybir.dt.uint32), 0)
        nc.scalar.dma_start(out=irows[2:4, :], in_=ilo[:, :])
        nc.scalar.dma_start(out=irows[4:5, :], in_=on_h[:][:, 0:N_IN])
        nc.gpsimd.dma_start(out=trows[2:4, :], in_=tlo[:, :])
        nc.sync.dma_start(out=trows[4:5, :], in_=on_h[:])

        # ---- input features: outer-product args -> range-reduce -> Sin ----
        with tcx.tile_pool(name="amI", bufs=4) as ami_pool:
            for it in range(NTI):
                psi = psa_pool.tile([128, F], F32, tag="psa")
                nc.tensor.matmul(psi[:, :], cmat[:, :],
                                 irows[:, it * F:(it + 1) * F],
                                 start=True, stop=True)
                t1 = ami_pool.tile([64, F], F32, tag="t1")
                am = ami_pool.tile([64, F], F32, tag="am")
                nc.scalar.activation(t1[:, :], psi[0:64, :], AF.Identity,
                                     bias=mgc[:, 0:1])
                nc.vector.scalar_tensor_tensor(am[:, :], t1[:, :], MAGIC,
                                               psi[0:64, :], op0=ALU.subtract,
                                               op1=ALU.subtract)
                nc.scalar.activation(fin_rows[:, it * F:(it + 1) * F],
                                     am[:, :], AF.Sin, scale=-TWO_PI)

        stage_cm.__exit__(None, None, None)

        # ---- transpose to nat layout, v-scale, M accumulation ----
        fin3 = fin_nat.rearrange("p (c w) -> p c w", w=128)
        psm = psm_pool.tile([128, 128], F32, tag="psm")
        cp_engs = [nc.scalar, nc.vector]
        with tcx.tile_pool(name="tp", bufs=3, space="PSUM") as tp_pool:
            for icc in range(IC):
                pst = tp_pool.tile([128, 128], F16, tag="tp")
                nc.tensor.transpose(pst[:, 0:64],
                                    fin_rows[:, icc * 128:(icc + 1) * 128],
                                    ident[0:64, 0:64])
                eng = cp_engs[icc % 2]
                if eng is nc.scalar:
                    nc.scalar.copy(fin3[:, icc, 64:128], pst[:, 0:64])
                else:
                    eng.tensor_copy(fin3[:, icc, 64:128], pst[:, 0:64])
                nc.vector.tensor_scalar(fin3[:, icc, 0:64],
                                        fin3[:, icc, 64:128],
                                        v_nat[:, icc:icc + 1], None,
                                        op0=ALU.mult)
                nc.tensor.matmul(psm[:, :], fin_nat[:, icc * 128:(icc + 1) * 128],
                                 fin_nat[:, icc * 128:(icc + 1) * 128],
                                 start=(icc == 0), stop=(icc == IC - 1))
            # M1/M0 -> transpose -> a_n scale -> lhsG
            nc.scalar.copy(msb[:, :], psm[:, 64:128])
            pst = tp_pool.tile([128, 128], F16, tag="tp")
            nc.tensor.transpose(pst[0:64, :], msb[:, :], ident[:, :])
            nc.vector.tensor_scalar(lhsg[0:64, :], pst[0:64, :],
                                    agy[:, 0:1], None, op0=ALU.mult)

        # ---- main loop over target column tiles ----
        with (
            tcx.tile_pool(name="red", bufs=3, space="PSUM") as red_pool,
            tcx.tile_pool(name="amT", bufs=4) as amt_pool,
            tcx.tile_pool(name="ft", bufs=3) as ft_pool,
            tcx.tile_pool(name="tt", bufs=3) as tt_pool,
            tcx.tile_pool(name="grp", bufs=2) as grp_pool,
        ):
            for ot in range(NT):
                psa = psa_pool.tile([128, F], F32, tag="psa")
                nc.tensor.matmul(psa[:, :], cmat[:, :],
                                 trows[:, ot * F:(ot + 1) * F],
                                 start=True, stop=True)
                t1 = amt_pool.tile([64, F], F32, tag="t1")
                am = amt_pool.tile([64, F], F32, tag="am")
                nc.scalar.activation(t1[:, :], psa[0:64, :], AF.Identity,
                                     bias=mgc[:, 0:1])
                nc.vector.scalar_tensor_tensor(am[:, :], t1[:, :], MAGIC,
                                               psa[0:64, :], op0=ALU.subtract,
                                               op1=ALU.subtract)
                ft = ft_pool.tile([128, F], F16, tag="ft")
                if ot < 3:  # ring has 3 buffers; zero the padded K rows once
                    nc.gpsimd.memset(ft[64:128, :].bitcast(mybir.dt.uint32), 0)
                nc.scalar.activation(ft[0:64, :], am[:, :], AF.Sin,
                                     scale=-TWO_PI)
                psg = psg_pool.tile([128, F], F32, tag="psg")
                nc.tensor.matmul(psg[:, :], lhsg[:, :], ft[:, :],
                                 start=True, stop=True)
                tt = tt_pool.tile([128, F], F16, tag="tt")
                nc.vector.tensor_mul(tt[0:64, :], ft[0:64, :], psg[0:64, :])
                nc.vector.tensor_mul(tt[64:128, :], ft[0:64, :], psg[64:128, :])
                psr = red_pool.tile([128, F], F32, tag="red")
                nc.tensor.matmul(psr[:, :], redt[:, :], tt[:, :],
                                 start=True, stop=True)
                nd_eng = nc.vector if ot % 2 == 0 else nc.scalar
                if nd_eng is nc.scalar:
                    nc.scalar.copy(nd_rows[:, ot * F:(ot + 1) * F],
                                   psr[0:33, :])
                else:
                    nd_eng.tensor_copy(nd_rows[:, ot * F:(ot + 1) * F],
                                       psr[0:33, :])
                if ot % 2 == 0:
                    continue

                # ---- finalize group g = ot//2: divide + x4 + x n_samples ----
                g = ot // 2
                gnum = grp_pool.tile([PG2, L], F32, tag="gnum")
                gden = grp_pool.tile([PG2, L], F32, tag="gden")
                grep = grp_pool.tile([PG2, 4 * L], F32, tag="grep")
                c0, c1 = g * 2 * F, (g + 1) * 2 * F
                nc.sync.dma_start(
                    out=gnum[:, :],
                    in_=nd_rows[0:1, c0:c1].rearrange("r (p k) -> r p k", k=L))
                nc.gpsimd.dma_start(
                    out=gden[:, :],
                    in_=nd_rows[32:33, c0:c1].rearrange("r (p k) -> r p k", k=L))
                nc.vector.tensor_scalar_add(gden[:, :], gden[:, :], EPS)
                nc.vector.reciprocal(gden[:, :], gden[:, :])
                nc.gpsimd.tensor_mul(gnum[:, :], gnum[:, :], gden[:, :])
                grep3 = grep.rearrange("p (k t) -> p k t", t=4)
                for t in range(4):
                    nc.gpsimd.tensor_copy(grep3[:, :, t], gnum[:, :])
                engs = [nc.sync, nc.gpsimd]
                for si in range(S):
                    engs[si % 2].dma_start(
                        out=out_h[:][si].rearrange("o t -> (o t)").rearrange(
                            "(p j) -> p j", p=128)[g * PG2:(g + 1) * PG2, :],
                        in_=grep[:, :])

        psm_cm.__exit__(None, None, None)
        psg_cm.__exit__(None, None, None)
        psa_cm.__exit__(None, None, None)


@lru_cache(maxsize=2)
def build_nc():
    nc = bacc.Bacc("TRN2", target_bir_lowering=False, debug=False)
    dat = nc.dram_tensor("dat", [T_IN, N_IN, V_IN], F32, kind="ExternalInput")
    ic_h = nc.dram_tensor("ic", [N_IN, 2], F32, kind="ExternalInput")
    tc_h = nc.dram_tensor("tc", [N_OUT, 2], F32, kind="ExternalInput")
    cm_h = nc.dram_tensor("cmat", [8, 128], F16, kind="ExternalInput")
    ag_h = nc.dram_tensor("agy", [64, 1], F32, kind="ExternalInput")
    rt_h = nc.dram_tensor("redt", [128, 128], F16, kind="ExternalInput")
    id_h = nc.dram_tensor("ident", [128, 128], F16, kind="ExternalInput")
    on_h = nc.dram_tensor("ones", [1, N_OUT], F16, kind="ExternalInput")
    mg_h = nc.dram_tensor("mgc", [64, 1], F32, kind="ExternalInput")
    out_h = nc.dram_tensor("out", [S, N_OUT, T_OUT], F32, kind="ExternalOutput")
    with tile.TileContext(nc) as tcx:
        build_kernel(tcx, dat, ic_h, tc_h, cm_h, ag_h, rt_h, id_h, on_h, mg_h, out_h)
    nc.compile()
    return nc


def _run(input_data, input_coords, target_coords, n_samples, trace=False):
    n_samples = int(n_samples)
    assert n_samples == S, f"kernel compiled for n_samples={S}, got {n_samples}"
    assert input_data.shape == (B, T_IN, N_IN, V_IN)
    nc = build_nc()
    cmat, agy, redt, ident = _consts()
    in_maps = [
        {
            "dat": np.ascontiguousarray(input_data[b], dtype=np.float32),
            "ic": np.ascontiguousarray(input_coords[b], dtype=np.float32),
            "tc": np.ascontiguousarray(target_coords[b], dtype=np.float32),
            "cmat": cmat,
            "agy": agy,
            "redt": redt,
            "ident": ident,
            "ones": np.ones((1, N_OUT), np.float16),
            "mgc": np.full((64, 1), MAGIC, np.float32),
        }
        for b in range(B)
    ]
    res = run_bass_kernel_spmd(nc, in_maps, list(range(B)), trace=trace)
    out = np.stack([res.results[b]["out"] for b in range(B)], axis=0)
    return out, res


def kernel(input_data, input_coords, target_coords, n_samples):
    out, _ = _run(
        np.asarray(input_data),
        np.asarray(input_coords),
        np.asarray(target_coords),
        n_samples,
    )
    return out


# revision 9
# speedup vs baseline: 2.7497x; 2.3768x over previous
"""RBF/KNN interpolation kernel for Trainium2 (8 NeuronCores, data parallel).

Algorithmic core: the Gaussian RBF kernel is separable and effectively
low-rank on [0,1]^2.  With sigma = 0.1,

    exp(-(a-b)^2 / (2 s^2)) = sum_m  a_m cos(pi m (a-b)),
    a_m = s sqrt(2 pi) exp(-(pi m s)^2 / 2)   (a_0 halved),

truncated at m <= 12 (error ~2e-5).  Expanding cos(pi m (a-b)) into
cos/sin products gives a rank-25-per-dimension feature map phi, and the
2-D kernel w[o,i] = kx * ky becomes a bilinear form:

    interp(o) = [phix(t_o)^T M1 phiy(t_o)] / [phix(t_o)^T M0 phiy(t_o)],
    M1 = sum_i v_i phix(x_i) phiy(x_i)^T,   M0 = same with v=1.

So instead of an 8192x4096 dense weight pass (33.5M exps), each core does
a few small matmuls over 64 feature slots:

  1. coords arrive nat-layout ([128, chunks] contiguous, fast DMA); the
     fp16 hi/lo split runs as wide 128-partition ops into an interleaved
     combo tile; [128, 8] PE transposes assemble the [8, N] rhs rows
     (xh, yh, xl, yl, ones) chunk by chunk,
  2. args u = (m/2) t + phase via a K=8 PE outer product (freqs m/2 are
     exactly fp16-representable),
  3. range-reduce u to [-0.5, 0.5) turns with the fp32 magic-number
     round trick (HW Sin is only accurate on [-pi, pi]),
  4. one Sin activation -> 64 feature rows per 512-target tile,
  5. G = lhsG^T ft  (lhsG folds a_n and M1/M0), T = ft .* G,
     num/den = redT^T T (redT folds a_m) -- all PE matmuls,
  6. divide in a [16, 64] nat layout, broadcast x4 and x n_samples
     to the output.

Slot layout (64): 0:13 x-cos m=0..12, 16:29 y-cos, 32:44 x-sin m=1..12,
48:60 y-sin, rest zero.  Coefficients enter only through the small
matmul operands (lhsG per-n, redT per-m), never the big feature tiles.
"""

from contextlib import ExitStack
from functools import lru_cache

import numpy as np

import concourse.bass as bass
import concourse.bacc as bacc
import concourse.tile as tile
from concourse import mybir
from concourse.bass_utils import run_bass_kernel_spmd

F32 = mybir.dt.float32
F16 = mybir.dt.float16
U32 = mybir.dt.uint32
AF = mybir.ActivationFunctionType
ALU = mybir.AluOpType

B = 8
T_IN = 4
N_IN = 4096
V_IN = 3
N_OUT = 8192
S = 10
T_OUT = 4
SIG = 0.1
EPS = 1e-8
MM = 12             # max cosine harmonic
MAGIC = 12582912.0  # 1.5 * 2^23: x + MAGIC - MAGIC == round(x) for |x| < 2^22
TWO_PI = 2.0 * np.pi

XCOS = list(range(0, 13))
YCOS = list(range(16, 29))
XSIN = list(range(32, 44))
YSIN = list(range(48, 60))


def _consts():
    def am(m):
        v = SIG * np.sqrt(2 * np.pi) * np.exp(-((np.pi * m * SIG) ** 2) / 2)
        return v / 2 if m == 0 else v

    cmat = np.zeros((8, 128), np.float16)
    for i, m in enumerate(range(0, MM + 1)):
        cmat[0, XCOS[i]] = m / 2.0
        cmat[2, XCOS[i]] = m / 2.0
        cmat[4, XCOS[i]] = 0.25
        cmat[1, YCOS[i]] = m / 2.0
        cmat[3, YCOS[i]] = m / 2.0
        cmat[4, YCOS[i]] = 0.25
    for i, m in enumerate(range(1, MM + 1)):
        cmat[0, XSIN[i]] = m / 2.0
        cmat[2, XSIN[i]] = m / 2.0
        cmat[1, YSIN[i]] = m / 2.0
        cmat[3, YSIN[i]] = m / 2.0

    agy = np.zeros((64, 1), np.float32)
    for i, m in enumerate(range(0, MM + 1)):
        agy[YCOS[i], 0] = am(m)
    for i, m in enumerate(range(1, MM + 1)):
        agy[YSIN[i], 0] = am(m)

    redt = np.zeros((128, 128), np.float16)
    for i, m in enumerate(range(0, MM + 1)):
        redt[XCOS[i], 0] = am(m)
        redt[64 + XCOS[i], 32] = am(m)
    for i, m in enumerate(range(1, MM + 1)):
        redt[XSIN[i], 0] = am(m)
        redt[64 + XSIN[i], 32] = am(m)

    ident = np.eye(128, dtype=np.float16)
    mgc = np.full((64, 1), MAGIC, np.float32)
    return cmat, agy, redt, ident, mgc


def build_kernel(tcx, tcn_h, icn_h, vn_h, cm_h, ag_h, rt_h, id_h, mg_h, out_h):
    nc = tcx.nc
    IC = N_IN // 128       # input point chunks
    TC = N_OUT // 128      # target point chunks
    F = 512
    NT = N_OUT // F        # main-loop column tiles
    NTI = N_IN // F        # input-side column tiles
    L = N_OUT // 128       # targets per output partition
    PG2 = 2 * F // L       # output partitions finalized per 2 tiles

    with ExitStack() as ctx:
        const_pool = ctx.enter_context(tcx.tile_pool(name="const", bufs=1))
        cmat = const_pool.tile([8, 128], F16)
        agy = const_pool.tile([64, 1], F32)
        mgc = const_pool.tile([64, 1], F32)
        redt = const_pool.tile([128, 128], F16)
        ident = const_pool.tile([128, 128], F16)
        tcn = const_pool.tile([128, 2 * TC], F32)
        icn = const_pool.tile([128, 2 * IC], F32)
        combo_t = const_pool.tile([128, 8 * TC], F16)
        combo_i = const_pool.tile([128, 8 * IC], F16)
        trows = const_pool.tile([8, N_OUT], F16)
        irows = const_pool.tile([8, N_IN], F16)
        v_nat = const_pool.tile([128, IC], F32)
        fin_rows = const_pool.tile([64, N_IN], F16)
        fin_nat = const_pool.tile([128, IC * 128], F16)
        msb = const_pool.tile([128, 64], F16)
        lhsg = const_pool.tile([128, 128], F16)
        nd_rows = const_pool.tile([33, N_OUT], F32)
        wsrc = const_pool.tile([128, 512], F16)

        psa_cm = tcx.tile_pool(name="psa", bufs=2, space="PSUM")
        psa_pool = psa_cm.__enter__()
        psg_cm = tcx.tile_pool(name="psg", bufs=2, space="PSUM")
        psg_pool = psg_cm.__enter__()
        tp_cm = tcx.tile_pool(name="tp", bufs=2, space="PSUM")
        tp_pool = tp_cm.__enter__()
        psm_cm = tcx.tile_pool(name="psm", bufs=1, space="PSUM")
        psm_pool = psm_cm.__enter__()

        # PE clock warm-up (HAM un-throttles only under sustained
        # full-array work); fills the head DMA window.
        nc.gpsimd.memset(wsrc[:, :].bitcast(U32), 0)
        for _ in range(20):
            wps = psg_pool.tile([128, F], F32, tag="psg")
            nc.tensor.matmul(wps[:, :], wsrc[:, 0:128], wsrc[:, :],
                             start=True, stop=True)

        # ---- head DMAs (all contiguous per partition) ----
        nc.sync.dma_start(out=cmat[:, :], in_=cm_h[:])
        nc.sync.dma_start(out=agy[:, :], in_=ag_h[:])
        nc.sync.dma_start(out=mgc[:, :], in_=mg_h[:])
        nc.gpsimd.dma_start(out=redt[:, :], in_=rt_h[:])
        nc.gpsimd.dma_start(out=ident[:, :], in_=id_h[:])
        nc.scalar.dma_start(out=icn[:, :], in_=icn_h[:])
        nc.scalar.dma_start(out=v_nat[:, :], in_=vn_h[:])
        nc.sync.dma_start(out=tcn[:, 0:TC], in_=tcn_h[:][:, 0:TC])
        nc.gpsimd.dma_start(out=tcn[:, TC:], in_=tcn_h[:][:, TC:])

        # ---- wide hi/lo fp16 splits into interleaved combo tiles ----
        # combo col c*8+r: r=0 xh, 1 yh, 2 xl, 3 yl, 4 ones, 5:8 zero
        c3t = combo_t.rearrange("p (c r) -> p c r", r=8)
        c3i = combo_i.rearrange("p (c r) -> p c r", r=8)
        nc.vector.memset(combo_t[:, :].bitcast(U32), 0)
        nc.gpsimd.memset(combo_i[:, :].bitcast(U32), 0)
        nc.vector.tensor_copy(c3t[:, :, 0], tcn[:, 0:TC])
        nc.vector.tensor_copy(c3t[:, :, 1], tcn[:, TC:])
        nc.vector.tensor_sub(c3t[:, :, 2], tcn[:, 0:TC], c3t[:, :, 0])
        nc.vector.tensor_sub(c3t[:, :, 3], tcn[:, TC:], c3t[:, :, 1])
        nc.vector.memset(c3t[:, :, 4], 1.0)
        nc.gpsimd.tensor_copy(c3i[:, :, 0], icn[:, 0:IC])
        nc.gpsimd.tensor_copy(c3i[:, :, 1], icn[:, IC:])
        nc.gpsimd.tensor_sub(c3i[:, :, 2], icn[:, 0:IC], c3i[:, :, 0])
        nc.gpsimd.tensor_sub(c3i[:, :, 3], icn[:, IC:], c3i[:, :, 1])
        nc.gpsimd.memset(c3i[:, :, 4], 1.0)
        nc.vector.memset(lhsg[64:128, :].bitcast(U32), 0)

        # ---- assemble rows tiles via [128, 8] PE transposes ----
        def rows_chunk(c, combo3, rows, eng_idx):
            psc = tp_pool.tile([128, 128], F16, tag="tp")
            nc.tensor.transpose(psc[0:8, :], combo3[:, c, :], ident[:, :])
            if eng_idx == 0:
                nc.scalar.copy(rows[:, c * 128:(c + 1) * 128], psc[0:8, :])
            else:
                nc.vector.tensor_copy(rows[:, c * 128:(c + 1) * 128],
                                      psc[0:8, :])

        for c in range(IC):
            rows_chunk(c, c3i, irows, c % 2)

        # ---- input features: outer-product args -> range-reduce -> Sin ----
        with tcx.tile_pool(name="amI", bufs=4) as ami_pool:
            for it in range(NTI):
                psi = psa_pool.tile([128, F], F32, tag="psa")
                nc.tensor.matmul(psi[:, :], cmat[:, :],
                                 irows[:, it * F:(it + 1) * F],
                                 start=True, stop=True)
                t1 = ami_pool.tile([64, F], F32, tag="t1")
                am = ami_pool.tile([64, F], F32, tag="am")
                nc.scalar.activation(t1[:, :], psi[0:64, :], AF.Identity,
                                     bias=mgc[:, 0:1])
                nc.vector.scalar_tensor_tensor(am[:, :], t1[:, :], MAGIC,
                                               psi[0:64, :], op0=ALU.subtract,
                                               op1=ALU.subtract)
                nc.scalar.activation(fin_rows[:, it * F:(it + 1) * F],
                                     am[:, :], AF.Sin, scale=-TWO_PI)

        # ---- transpose to nat layout, v-scale, M accumulation ----
        fin3 = fin_nat.rearrange("p (c w) -> p c w", w=128)
        psm = psm_pool.tile([128, 128], F32, tag="psm")
        for icc in range(IC):
            pst = tp_pool.tile([128, 128], F16, tag="tp")
            nc.tensor.transpose(pst[:, 0:64],
                                fin_rows[:, icc * 128:(icc + 1) * 128],
                                ident[0:64, 0:64])
            if icc % 2 == 0:
                nc.scalar.copy(fin3[:, icc, 64:128], pst[:, 0:64])
            else:
                nc.vector.tensor_copy(fin3[:, icc, 64:128], pst[:, 0:64])
            nc.vector.tensor_scalar(fin3[:, icc, 0:64],
                                    fin3[:, icc, 64:128],
                                    v_nat[:, icc:icc + 1], None,
                                    op0=ALU.mult)
            nc.tensor.matmul(psm[:, :], fin_nat[:, icc * 128:(icc + 1) * 128],
                             fin_nat[:, icc * 128:(icc + 1) * 128],
                             start=(icc == 0), stop=(icc == IC - 1))
        # M1/M0 -> transpose -> a_n scale -> lhsG
        nc.scalar.copy(msb[:, :], psm[:, 64:128])
        psm_cm.__exit__(None, None, None)
        pst = tp_pool.tile([128, 128], F16, tag="tp")
        nc.tensor.transpose(pst[0:64, :], msb[:, :], ident[:, :])
        nc.vector.tensor_scalar(lhsg[0:64, :], pst[0:64, :],
                                agy[:, 0:1], None, op0=ALU.mult)

        # ---- main loop over target column tiles ----
        with (
            tcx.tile_pool(name="red", bufs=2, space="PSUM") as red_pool,
            tcx.tile_pool(name="amT", bufs=4) as amt_pool,
            tcx.tile_pool(name="ft", bufs=3) as ft_pool,
            tcx.tile_pool(name="tt", bufs=3) as tt_pool,
            tcx.tile_pool(name="grp", bufs=2) as grp_pool,
        ):
            CPT = TC // NT  # target chunks per tile
            for ot in range(NT):
                for j in range(CPT):
                    rows_chunk(ot * CPT + j, c3t, trows, j % 2)
                psa = psa_pool.tile([128, F], F32, tag="psa")
                nc.tensor.matmul(psa[:, :], cmat[:, :],
                                 trows[:, ot * F:(ot + 1) * F],
                                 start=True, stop=True)
                t1 = amt_pool.tile([64, F], F32, tag="t1")
                am = amt_pool.tile([64, F], F32, tag="am")
                nc.scalar.activation(t1[:, :], psa[0:64, :], AF.Identity,
                                     bias=mgc[:, 0:1])
                nc.vector.scalar_tensor_tensor(am[:, :], t1[:, :], MAGIC,
                                               psa[0:64, :], op0=ALU.subtract,
                                               op1=ALU.subtract)
                ft = ft_pool.tile([128, F], F16, tag="ft")
                if ot < 3:  # ring has 3 buffers; zero the padded K rows once
                    nc.gpsimd.memset(ft[64:128, :].bitcast(U32), 0)
                nc.scalar.activation(ft[0:64, :], am[:, :], AF.Sin,
                                     scale=-TWO_PI)
                psg = psg_pool.tile([128, F], F32, tag="psg")
                nc.tensor.matmul(psg[:, :], lhsg[:, :], ft[:, :],
                                 start=True, stop=True)
                tt = tt_pool.tile([128, F], F16, tag="tt")
                nc.vector.tensor_mul(tt[0:64, :], ft[0:64, :], psg[0:64, :])
                nc.vector.tensor_mul(tt[64:128, :], ft[0:64, :],
                                     psg[64:128, :])
                psr = red_pool.tile([128, F], F32, tag="red")
                nc.tensor.matmul(psr[:, :], redt[:, :], tt[:, :],
                                 start=True, stop=True)
                if ot % 2 == 0:
                    nc.vector.tensor_copy(nd_rows[:, ot * F:(ot + 1) * F],
                                          psr[0:33, :])
                    continue
                nc.scalar.copy(nd_rows[:, ot * F:(ot + 1) * F], psr[0:33, :])

                # ---- finalize group g: divide + x4 + x n_samples ----
                g = ot // 2
                gnum = grp_pool.tile([PG2, L], F32, tag="gnum")
                gden = grp_pool.tile([PG2, L], F32, tag="gden")
                grep = grp_pool.tile([PG2, 4 * L], F32, tag="grep")
                c0, c1 = g * 2 * F, (g + 1) * 2 * F
                nc.sync.dma_start(
                    out=gnum[:, :],
                    in_=nd_rows[0:1, c0:c1].rearrange("r (p k) -> r p k", k=L))
                nc.gpsimd.dma_start(
                    out=gden[:, :],
                    in_=nd_rows[32:33, c0:c1].rearrange("r (p k) -> r p k", k=L))
                nc.vector.tensor_scalar_add(gden[:, :], gden[:, :], EPS)
                nc.vector.reciprocal(gden[:, :], gden[:, :])
                nc.gpsimd.tensor_mul(gnum[:, :], gnum[:, :], gden[:, :])
                grep3 = grep.rearrange("p (k t) -> p k t", t=4)
                for t in range(4):
                    nc.gpsimd.tensor_copy(grep3[:, :, t], gnum[:, :])
                engs = [nc.sync, nc.gpsimd]
                for si in range(S):
                    engs[si % 2].dma_start(
                        out=out_h[:][si].rearrange("o t -> (o t)").rearrange(
                            "(p j) -> p j", p=128)[g * PG2:(g + 1) * PG2, :],
                        in_=grep[:, :])

        tp_cm.__exit__(None, None, None)
        psg_cm.__exit__(None, None, None)
        psa_cm.__exit__(None, None, None)


@lru_cache(maxsize=2)
def build_nc():
    nc = bacc.Bacc("TRN2", target_bir_lowering=False, debug=False)
    tcn_h = nc.dram_tensor("tcn", [128, N_OUT // 64], F32, kind="ExternalInput")
    icn_h = nc.dram_tensor("icn", [128, N_IN // 64], F32, kind="ExternalInput")
    vn_h = nc.dram_tensor("vn", [128, N_IN // 128], F32, kind="ExternalInput")
    cm_h = nc.dram_tensor("cmat", [8, 128], F16, kind="ExternalInput")
    ag_h = nc.dram_tensor("agy", [64, 1], F32, kind="ExternalInput")
    rt_h = nc.dram_tensor("redt", [128, 128], F16, kind="ExternalInput")
    id_h = nc.dram_tensor("ident", [128, 128], F16, kind="ExternalInput")
    mg_h = nc.dram_tensor("mgc", [64, 1], F32, kind="ExternalInput")
    out_h = nc.dram_tensor("out", [S, N_OUT, T_OUT], F32, kind="ExternalOutput")
    with tile.TileContext(nc) as tcx:
        build_kernel(tcx, tcn_h, icn_h, vn_h, cm_h, ag_h, rt_h, id_h, mg_h,
                     out_h)
    nc.compile()
    return nc


def _nat(a, chunks):
    # [N] -> [128, chunks] with nat[p, c] = a[c*128 + p]
    return np.ascontiguousarray(a.reshape(chunks, 128).T)


def _run(input_data, input_coords, target_coords, n_samples, trace=False):
    n_samples = int(n_samples)
    assert n_samples == S, f"kernel compiled for n_samples={S}, got {n_samples}"
    assert input_data.shape == (B, T_IN, N_IN, V_IN)
    nc = build_nc()
    cmat, agy, redt, ident, mgc = _consts()
    in_maps = []
    for b in range(B):
        tc = np.asarray(target_coords[b], dtype=np.float32)
        ic = np.asarray(input_coords[b], dtype=np.float32)
        v = np.asarray(input_data[b, T_IN - 1, :, 0], dtype=np.float32)
        in_maps.append({
            "tcn": np.hstack([_nat(tc[:, 0], N_OUT // 128),
                              _nat(tc[:, 1], N_OUT // 128)]),
            "icn": np.hstack([_nat(ic[:, 0], N_IN // 128),
                              _nat(ic[:, 1], N_IN // 128)]),
            "vn": _nat(v, N_IN // 128),
            "cmat": cmat,
            "agy": agy,
            "redt": redt,
            "ident": ident,
            "mgc": mgc,
        })
    res = run_bass_kernel_spmd(nc, in_maps, list(range(B)), trace=trace)
    out = np.stack([res.results[b]["out"] for b in range(B)], axis=0)
    return out, res


def kernel(input_data, input_coords, target_coords, n_samples):
    out, _ = _run(
        np.asarray(input_data),
        np.asarray(input_coords),
        np.asarray(target_coords),
        n_samples,
    )
    return out


# revision 12
# speedup vs baseline: 3.2018x; 1.1644x over previous
"""RBF/KNN interpolation kernel for Trainium2 (8 NeuronCores, data parallel).

Algorithmic core: the Gaussian RBF kernel is separable and effectively
low-rank on [0,1]^2.  With sigma = 0.1,

    exp(-(a-b)^2 / (2 s^2)) = sum_m  a_m cos(pi m (a-b)),
    a_m = s sqrt(2 pi) exp(-(pi m s)^2 / 2)   (a_0 halved),

truncated at m <= 12 (error ~2e-5).  Expanding cos(pi m (a-b)) into
cos/sin products gives a rank-25-per-dimension feature map phi, and the
2-D kernel w[o,i] = kx * ky becomes a bilinear form:

    interp(o) = [phix(t_o)^T M1 phiy(t_o)] / [phix(t_o)^T M0 phiy(t_o)],
    M1 = sum_i v_i phix(x_i) phiy(x_i)^T,   M0 = same with v=1.

So instead of an 8192x4096 dense weight pass (33.5M exps), each core does
a few small matmuls over 64 feature slots:

  1. coords arrive nat-layout ([128, chunks] contiguous, fast DMA); the
     fp16 hi/lo split runs as wide 128-partition ops into a stride-32
     interleaved combo tile; batched [128, 128] PE transposes (4 chunks
     at 32-aligned offsets) yield [8, 128] component chunks,
  2. target args u = (m/2) t + phase via K=8 PE outer products; input
     args computed directly in nat layout ([8,128] chunk as lhsT),
  3. range-reduce u to [-0.5, 0.5) turns with the fp32 magic-number
     round trick (HW Sin is only accurate on [-pi, pi]), one Sin
     activation per tile; target ft rows 64:128 duplicate 0:64 so the
     T-multiply is a single [128, F] op,
  4. G = lhsG^T ft  (lhsG folds a_n and M1/M0), T = ft .* G,
     num/den = redT^T T (redT folds a_m) -- all PE matmuls,
  5. divide in a [16, 64] nat layout, broadcast x4, write sample 0,
     then 9 contiguous 128KB DRAM->DRAM copies for the other samples.

Slot layout (64): 0:13 x-cos m=0..12, 16:29 y-cos, 32:44 x-sin m=1..12,
48:60 y-sin, rest zero.  Coefficients enter only through the small
matmul operands (lhsG per-n, redT per-m), never the big feature tiles.
"""

from contextlib import ExitStack
from functools import lru_cache

import numpy as np

import concourse.bass as bass
import concourse.bacc as bacc
import concourse.tile as tile
from concourse import mybir
from concourse.bass_utils import run_bass_kernel_spmd

F32 = mybir.dt.float32
F16 = mybir.dt.float16
U32 = mybir.dt.uint32
AF = mybir.ActivationFunctionType
ALU = mybir.AluOpType

B = 8
T_IN = 4
N_IN = 4096
V_IN = 3
N_OUT = 8192
S = 10
T_OUT = 4
SIG = 0.1
EPS = 1e-8
MM = 12             # max cosine harmonic
MAGIC = 12582912.0  # 1.5 * 2^23: x + MAGIC - MAGIC == round(x) for |x| < 2^22
TWO_PI = 2.0 * np.pi

XCOS = list(range(0, 13))
YCOS = list(range(16, 29))
XSIN = list(range(32, 44))
YSIN = list(range(48, 60))


def _consts():
    def am(m):
        v = SIG * np.sqrt(2 * np.pi) * np.exp(-((np.pi * m * SIG) ** 2) / 2)
        return v / 2 if m == 0 else v

    cmat = np.zeros((8, 128), np.float16)
    for i, m in enumerate(range(0, MM + 1)):
        cmat[0, XCOS[i]] = m / 2.0
        cmat[2, XCOS[i]] = m / 2.0
        cmat[4, XCOS[i]] = 0.25
        cmat[1, YCOS[i]] = m / 2.0
        cmat[3, YCOS[i]] = m / 2.0
        cmat[4, YCOS[i]] = 0.25
    for i, m in enumerate(range(1, MM + 1)):
        cmat[0, XSIN[i]] = m / 2.0
        cmat[2, XSIN[i]] = m / 2.0
        cmat[1, YSIN[i]] = m / 2.0
        cmat[3, YSIN[i]] = m / 2.0

    agy = np.zeros((64, 1), np.float32)
    for i, m in enumerate(range(0, MM + 1)):
        agy[YCOS[i], 0] = am(m)
    for i, m in enumerate(range(1, MM + 1)):
        agy[YSIN[i], 0] = am(m)

    redt = np.zeros((128, 128), np.float16)
    for i, m in enumerate(range(0, MM + 1)):
        redt[XCOS[i], 0] = am(m)
        redt[64 + XCOS[i], 32] = am(m)
    for i, m in enumerate(range(1, MM + 1)):
        redt[XSIN[i], 0] = am(m)
        redt[64 + XSIN[i], 32] = am(m)

    ident = np.eye(128, dtype=np.float16)
    mgc = np.full((128, 1), MAGIC, np.float32)
    return cmat, agy, redt, ident, mgc


def build_kernel(tcx, tcn_h, icn_h, vn_h, cm_h, ag_h, rt_h, id_h, mg_h, out_h):
    nc = tcx.nc
    IC = N_IN // 128       # input point chunks
    TC = N_OUT // 128      # target point chunks
    F = 512
    NT = N_OUT // F        # main-loop column tiles
    L = N_OUT // 128       # targets per output partition
    PG2 = 2 * F // L       # output partitions finalized per 2 tiles

    with ExitStack() as ctx:
        const_pool = ctx.enter_context(tcx.tile_pool(name="const", bufs=1))
        cmat = const_pool.tile([8, 128], F16)
        agy = const_pool.tile([64, 1], F32)
        mgc = const_pool.tile([128, 1], F32)
        redt = const_pool.tile([128, 128], F16)
        ident = const_pool.tile([128, 128], F16)
        tcn = const_pool.tile([128, 2 * TC], F32)
        icn = const_pool.tile([128, 2 * IC], F32)
        combo_t = const_pool.tile([128, 32 * TC], F16)
        combo_i = const_pool.tile([128, 32 * IC], F16)
        trows = const_pool.tile([8, N_OUT], F16)
        irows = const_pool.tile([8, N_IN], F16)
        v_nat = const_pool.tile([128, IC], F32)
        fin_nat = const_pool.tile([128, IC * 128], F16)
        msb = const_pool.tile([128, 64], F16)
        lhsg = const_pool.tile([128, 128], F16)
        nd_rows = const_pool.tile([33, N_OUT], F32)
        wsrc = const_pool.tile([128, 512], F16)

        psa_cm = tcx.tile_pool(name="psa", bufs=2, space="PSUM")
        psa_pool = psa_cm.__enter__()
        psg_cm = tcx.tile_pool(name="psg", bufs=2, space="PSUM")
        psg_pool = psg_cm.__enter__()
        tp_cm = tcx.tile_pool(name="tp", bufs=2, space="PSUM")
        tp_pool = tp_cm.__enter__()
        psm_cm = tcx.tile_pool(name="psm", bufs=1, space="PSUM")
        psm_pool = psm_cm.__enter__()

        # PE clock warm-up: HAM un-throttles only under sustained
        # full-array work; fill the head DMA window.
        nc.gpsimd.memset(wsrc[:, :].bitcast(U32), 0)
        for _ in range(12):
            wps = psg_pool.tile([128, F], F32, tag="psg")
            nc.tensor.matmul(wps[:, :], wsrc[:, 0:128], wsrc[:, :],
                             start=True, stop=True)

        # ---- head DMAs (all contiguous per partition) ----
        nc.sync.dma_start(out=cmat[:, :], in_=cm_h[:])
        nc.sync.dma_start(out=agy[:, :], in_=ag_h[:])
        nc.sync.dma_start(out=mgc[:, :], in_=mg_h[:])
        nc.gpsimd.dma_start(out=redt[:, :], in_=rt_h[:])
        nc.gpsimd.dma_start(out=ident[:, :], in_=id_h[:])
        nc.scalar.dma_start(out=icn[:, :], in_=icn_h[:])
        nc.scalar.dma_start(out=v_nat[:, :], in_=vn_h[:])
        nc.sync.dma_start(out=tcn[:, 0:TC], in_=tcn_h[:][:, 0:TC])
        nc.gpsimd.dma_start(out=tcn[:, TC:], in_=tcn_h[:][:, TC:])

        # ---- wide hi/lo fp16 splits into stride-32 combo tiles ----
        # combo col c*32+r: r=0 xh, 1 yh, 2 xl, 3 yl, 4 ones, 5:32 zero
        c3t = combo_t.rearrange("p (c r) -> p c r", r=32)
        c3i = combo_i.rearrange("p (c r) -> p c r", r=32)
        nc.vector.memset(combo_t[:, :].bitcast(U32), 0)
        nc.gpsimd.memset(combo_i[:, :].bitcast(U32), 0)
        nc.vector.tensor_copy(c3t[:, :, 0], tcn[:, 0:TC])
        nc.vector.tensor_copy(c3t[:, :, 1], tcn[:, TC:])
        nc.vector.tensor_sub(c3t[:, :, 2], tcn[:, 0:TC], c3t[:, :, 0])
        nc.vector.tensor_sub(c3t[:, :, 3], tcn[:, TC:], c3t[:, :, 1])
        nc.vector.memset(c3t[:, :, 4], 1.0)
        nc.gpsimd.tensor_copy(c3i[:, :, 0], icn[:, 0:IC])
        nc.gpsimd.tensor_copy(c3i[:, :, 1], icn[:, IC:])
        nc.gpsimd.tensor_sub(c3i[:, :, 2], icn[:, 0:IC], c3i[:, :, 0])
        nc.gpsimd.tensor_sub(c3i[:, :, 3], icn[:, IC:], c3i[:, :, 1])
        nc.gpsimd.memset(c3i[:, :, 4], 1.0)
        nc.vector.memset(lhsg[64:128, :].bitcast(U32), 0)

        # ---- batched component transposes: 4 chunks per [128, 128] ----
        def rows_batch(c0, combo, rows, nchunks):
            psc = tp_pool.tile([128, 128], F16, tag="tp")
            nc.tensor.transpose(psc[:, :],
                                combo[:, c0 * 32:(c0 + 4) * 32],
                                ident[:, :])
            for j in range(min(4, nchunks - c0)):
                c = c0 + j
                if c % 2 == 0:
                    nc.scalar.copy(rows[:, c * 128:(c + 1) * 128],
                                   psc[32 * j:32 * j + 8, :])
                else:
                    nc.vector.tensor_copy(rows[:, c * 128:(c + 1) * 128],
                                          psc[32 * j:32 * j + 8, :])

        for c0 in range(0, IC, 4):
            rows_batch(c0, combo_i, irows, IC)

        # ---- input features direct in nat layout ----
        with tcx.tile_pool(name="amI", bufs=2) as ami_pool:
            NGI = IC // 8
            for g in range(NGI):
                psi = psa_pool.tile([128, F], F32, tag="psa")
                for j in range(8):
                    icc = g * 8 + j
                    nc.tensor.matmul(psi[:, j * 64:(j + 1) * 64],
                                     irows[:, icc * 128:(icc + 1) * 128],
                                     cmat[:, 0:64], start=True, stop=True)
                t1 = ami_pool.tile([128, F], F32, tag="t1")
                am = ami_pool.tile([128, F], F32, tag="am")
                nc.scalar.activation(t1[:, :], psi[:, :], AF.Identity,
                                     bias=mgc[:, 0:1])
                nc.vector.scalar_tensor_tensor(am[:, :], t1[:, :], MAGIC,
                                               psi[:, :], op0=ALU.subtract,
                                               op1=ALU.subtract)
                fslice = fin_nat.rearrange("p (c w) -> p c w", w=128)[
                    :, g * 8:(g + 1) * 8, 64:128]
                nc.scalar.activation(fslice, am[:, :], AF.Sin, scale=-TWO_PI)

        # ---- target component chunks ----
        for c0 in range(0, TC, 4):
            rows_batch(c0, combo_t, trows, TC)

        # ---- v-scale + M accumulation ----
        fin3 = fin_nat.rearrange("p (c w) -> p c w", w=128)
        psm = psm_pool.tile([128, 128], F32, tag="psm")
        for icc in range(IC):
            eng = nc.vector if icc % 2 == 0 else nc.gpsimd
            eng.tensor_scalar(fin3[:, icc, 0:64], fin3[:, icc, 64:128],
                              v_nat[:, icc:icc + 1], None, op0=ALU.mult)
            nc.tensor.matmul(psm[:, :], fin_nat[:, icc * 128:(icc + 1) * 128],
                             fin_nat[:, icc * 128:(icc + 1) * 128],
                             start=(icc == 0), stop=(icc == IC - 1))
        # M1/M0 -> transpose -> a_n scale -> lhsG
        nc.scalar.copy(msb[:, :], psm[:, 64:128])
        psm_cm.__exit__(None, None, None)
        pst = tp_pool.tile([128, 128], F16, tag="tp")
        nc.tensor.transpose(pst[0:64, :], msb[:, :], ident[:, :])
        nc.vector.tensor_scalar(lhsg[0:64, :], pst[0:64, :],
                                agy[:, 0:1], None, op0=ALU.mult)

        # ---- target features: outer product -> RR -> Sin (all tiles) ----
        with (
            tcx.tile_pool(name="red", bufs=2, space="PSUM") as red_pool,
            tcx.tile_pool(name="amT", bufs=3) as amt_pool,
            tcx.tile_pool(name="ft", bufs=16) as ft_pool,
            tcx.tile_pool(name="tt", bufs=3) as tt_pool,
            tcx.tile_pool(name="grp", bufs=2) as grp_pool,
        ):
            fts = []
            for ot in range(NT):
                psa = psa_pool.tile([128, F], F32, tag="psa")
                nc.tensor.matmul(psa[:, :], cmat[:, :],
                                 trows[:, ot * F:(ot + 1) * F],
                                 start=True, stop=True)
                t1 = amt_pool.tile([64, F], F32, tag="t1")
                am = amt_pool.tile([64, F], F32, tag="am")
                nc.vector.tensor_scalar_add(t1[:, :], psa[0:64, :], MAGIC)
                nc.vector.scalar_tensor_tensor(am[:, :], t1[:, :], MAGIC,
                                               psa[0:64, :], op0=ALU.subtract,
                                               op1=ALU.subtract)
                ft = ft_pool.tile([128, F], F16, tag="ft")
                nc.scalar.activation(ft[0:64, :], am[:, :], AF.Sin,
                                     scale=-TWO_PI)
                # rows 64:128 duplicate 0:64 so T-mult is one [128, F] op
                nc.gpsimd.tensor_copy(ft[64:128, :], ft[0:64, :])
                fts.append(ft)

            # ---- main loop: G, T, reduce, finalize ----
            for ot in range(NT):
                ft = fts[ot]
                psg = psg_pool.tile([128, F], F32, tag="psg")
                nc.tensor.matmul(psg[:, :], lhsg[:, :], ft[:, :],
                                 start=True, stop=True)
                tt = tt_pool.tile([128, F], F16, tag="tt")
                nc.vector.tensor_mul(tt[:, :], ft[:, :], psg[:, :])
                psr = red_pool.tile([128, F], F32, tag="red")
                nc.tensor.matmul(psr[:, :], redt[:, :], tt[:, :],
                                 start=True, stop=True)
                if ot % 2 == 0:
                    nc.scalar.copy(nd_rows[:, ot * F:(ot + 1) * F],
                                   psr[0:33, :])
                    continue
                nc.vector.tensor_copy(nd_rows[:, ot * F:(ot + 1) * F],
                                      psr[0:33, :])

                # ---- finalize group g: divide + x4 + sample-0 write ----
                g = ot // 2
                gnum = grp_pool.tile([PG2, L], F32, tag="gnum")
                gden = grp_pool.tile([PG2, L], F32, tag="gden")
                grep = grp_pool.tile([PG2, 4 * L], F32, tag="grep")
                c0, c1 = g * 2 * F, (g + 1) * 2 * F
                nc.sync.dma_start(
                    out=gnum[:, :],
                    in_=nd_rows[0:1, c0:c1].rearrange("r (p k) -> r p k", k=L))
                nc.gpsimd.dma_start(
                    out=gden[:, :],
                    in_=nd_rows[32:33, c0:c1].rearrange("r (p k) -> r p k", k=L))
                nc.vector.tensor_scalar_add(gden[:, :], gden[:, :], EPS)
                nc.vector.reciprocal(gden[:, :], gden[:, :])
                nc.gpsimd.tensor_mul(gnum[:, :], gnum[:, :], gden[:, :])
                grep3 = grep.rearrange("p (k t) -> p k t", t=4)
                for t in range(4):
                    nc.gpsimd.tensor_copy(grep3[:, :, t], gnum[:, :])
                nc.sync.dma_start(
                    out=out_h[:][0].rearrange("o t -> (o t)").rearrange(
                        "(p j) -> p j", p=128)[g * PG2:(g + 1) * PG2, :],
                    in_=grep[:, :])

            # ---- broadcast sample 0 to samples 1..9 (contiguous 128KB) ----
            engs = [nc.sync, nc.gpsimd, nc.scalar]
            src = out_h[:][0].rearrange("o t -> (o t)")
            for si in range(1, S):
                engs[si % 3].dma_start(
                    out=out_h[:][si].rearrange("o t -> (o t)"), in_=src)

        tp_cm.__exit__(None, None, None)
        psg_cm.__exit__(None, None, None)
        psa_cm.__exit__(None, None, None)


@lru_cache(maxsize=2)
def build_nc():
    nc = bacc.Bacc("TRN2", target_bir_lowering=False, debug=False)
    tcn_h = nc.dram_tensor("tcn", [128, N_OUT // 64], F32, kind="ExternalInput")
    icn_h = nc.dram_tensor("icn", [128, N_IN // 64], F32, kind="ExternalInput")
    vn_h = nc.dram_tensor("vn", [128, N_IN // 128], F32, kind="ExternalInput")
    cm_h = nc.dram_tensor("cmat", [8, 128], F16, kind="ExternalInput")
    ag_h = nc.dram_tensor("agy", [64, 1], F32, kind="ExternalInput")
    rt_h = nc.dram_tensor("redt", [128, 128], F16, kind="ExternalInput")
    id_h = nc.dram_tensor("ident", [128, 128], F16, kind="ExternalInput")
    mg_h = nc.dram_tensor("mgc", [128, 1], F32, kind="ExternalInput")
    out_h = nc.dram_tensor("out", [S, N_OUT, T_OUT], F32, kind="ExternalOutput")
    with tile.TileContext(nc) as tcx:
        build_kernel(tcx, tcn_h, icn_h, vn_h, cm_h, ag_h, rt_h, id_h, mg_h,
                     out_h)
    nc.compile()
    return nc


def _nat(a, chunks):
    # [N] -> [128, chunks] with nat[p, c] = a[c*128 + p]
    return np.ascontiguousarray(a.reshape(chunks, 128).T)


def _run(input_data, input_coords, target_coords, n_samples, trace=False):
    n_samples = int(n_samples)
    assert n_samples == S, f"kernel compiled for n_samples={S}, got {n_samples}"
    assert input_data.shape == (B, T_IN, N_IN, V_IN)
    nc = build_nc()
    cmat, agy, redt, ident, mgc = _consts()
    in_maps = []
    for b in range(B):
        tc = np.asarray(target_coords[b], dtype=np.float32)
        ic = np.asarray(input_coords[b], dtype=np.float32)
        v = np.asarray(input_data[b, T_IN - 1, :, 0], dtype=np.float32)
        in_maps.append({
            "tcn": np.hstack([_nat(tc[:, 0], N_OUT // 128),
                              _nat(tc[:, 1], N_OUT // 128)]),
            "icn": np.hstack([_nat(ic[:, 0], N_IN // 128),
                              _nat(ic[:, 1], N_IN // 128)]),
            "vn": _nat(v, N_IN // 128),
            "cmat": cmat,
            "agy": agy,
            "redt": redt,
            "ident": ident,
            "mgc": mgc,
        })
    res = run_bass_kernel_spmd(nc, in_maps, list(range(B)), trace=trace)
    out = np.stack([res.results[b]["out"] for b in range(B)], axis=0)
    return out, res


def kernel(input_data, input_coords, target_coords, n_samples):
    out, _ = _run(
        np.asarray(input_data),
        np.asarray(input_coords),
        np.asarray(target_coords),
        n_samples,
    )
    return out
